# revision 24
# baseline (speedup 1.0000x reference)
"""Multi-head causal attention (b=4, l=2048, d=1024, 16 heads x 64) on 8 trn2 cores.

Sharding: core c handles batch (c // 2) and head-group (c % 2) of 8 heads.
Each core computes a partial output x[b] @ W (its 8 heads' contribution);
the host sums the two partials per batch.

v2 design (vs fp32r baseline):
  - full bf16 datapath (PSUM stays fp32): enables PE fast-weight-load and
    halves DMA/SBUF traffic; accuracy is well within the 2e-2 gate.
  - single fused instruction stream: projections for l-chunk lc=j+1 and the
    output projection for q-block j-1 are interleaved (via a filler FIFO)
    into attention block j's S/PV matmuls, keeping the PE dense so it holds
    the fast HAM p-state and hides the exp (ACT) latency.
  - S tiles for the two heads of a pair land in one 2-bank PSUM tile and are
    exp'd with ONE activation instruction (halves ACT instruction overhead).
  - causal masking via gpsimd.affine_select on the diagonal 128x128 block of
    P (post-exp, fill=0); off-diagonal-block columns are skipped entirely
    (S / exp / PV all shortened to the unmasked column span).
  - softmax denominators via the [V|1] ones-column trick; normalization uses
    reciprocal_approx_fast (the exact DVE reciprocal is ~3.3us per call).

Device layouts (per core):
  xT      [1024, 2048]  x[b]^T (d on partitions), 32 sbuf tiles [128,512]
  wq/wk/wv [1024, 512]  head-group column slices; wo [512, 1024] row slice
  qT/kT   4 x [128, 2048]  c on partitions (head-pair per tile)
  vp      16 x [128, 8, 65]  v natural (l on partitions), per head 64 + ones
  OF      4 x [128, 2048]  normalized attention output (pre-out-proj)
  S^T     [128 m, 2x512 q] psum pairs -> exp -> P^T bf16; PV: [V|1]^T P^T
  q=0 is fully masked; its softmax sum is 0 and the garbage column is fixed
  up on the host (row 0 of the output only).
"""

import sys

sys.path.insert(0, "/opt/trn_rl_repo")

import ml_dtypes
import numpy as np

import concourse.bacc as bacc
import concourse.mybir as mybir
import concourse.tile as tile
from concourse.bass_utils import run_bass_kernel_spmd

F32 = mybir.dt.float32
BF16 = mybir.dt.bfloat16
AF = mybir.ActivationFunctionType
ALU = mybir.AluOpType

B, L, D = 4, 2048, 1024
N_HEAD, KEY_DIM = 16, 64
HG = 8               # heads per core (head-group)
C = HG * KEY_DIM     # 512 per-core qkv width
SCALE = 1.0 / 8.0    # 1/sqrt(KEY_DIM)
ND = 8               # d chunks of 128
NJ = 4               # q blocks of 512
NCC = 4              # c chunks of 128 (= head pairs)

_CACHED = {}


def act_reciprocal(nc, scratch, out, in_):
    """Reciprocal on the ACT engine as exp(-ln(x)).

    Ln and Exp share one activation-table set (natural_log_exp_and_others),
    so this never triggers a 1.3us ACT_TABLE_LOAD — unlike func=Reciprocal,
    whose table set excludes Exp and thrashes the table on every call. The
    DVE alternatives are unusable here: exact `reciprocal` costs ~3.3us per
    call (free-size-bound iterative), and `reciprocal_approx_fast`
    (custom-DVE uop table) returns garbage on HW via this compile path.
    """
    nc.scalar.activation(scratch, in_, AF.Ln)
    nc.scalar.activation(out, scratch, AF.Exp, scale=-1.0)


def build_nc():
    nc = bacc.Bacc("TRN2", target_bir_lowering=False, debug=False)

    xT = nc.dram_tensor("xT", [D, L], BF16, kind="ExternalInput")
    wq = nc.dram_tensor("wq", [D, C], BF16, kind="ExternalInput")
    wk = nc.dram_tensor("wk", [D, C], BF16, kind="ExternalInput")
    wv = nc.dram_tensor("wv", [D, C], BF16, kind="ExternalInput")
    wo = nc.dram_tensor("wo", [C, D], BF16, kind="ExternalInput")
    out = nc.dram_tensor("out", [L, D], F32, kind="ExternalOutput")

    with tile.TileContext(nc) as tc:
        with tc.tile_pool(name="wp", bufs=1) as wp, \
             tc.tile_pool(name="xp", bufs=1) as xp, \
             tc.tile_pool(name="qkv", bufs=1) as qkv, \
             tc.tile_pool(name="ofp", bufs=1) as ofp, \
             tc.tile_pool(name="pp", bufs=6) as pp, \
             tc.tile_pool(name="ovp", bufs=4) as ovp, \
             tc.tile_pool(name="bcp", bufs=2) as bcp, \
             tc.tile_pool(name="osb", bufs=2) as osb, \
             tc.tile_pool(name="psS", bufs=2, space="PSUM") as psS, \
             tc.tile_pool(name="psX", bufs=1, space="PSUM") as psX, \
             tc.tile_pool(name="psO", bufs=3, space="PSUM") as psO:

            wq_sb = [wp.tile([128, C], BF16, name=f"wq{d}") for d in range(ND)]
            wk_sb = [wp.tile([128, C], BF16, name=f"wk{d}") for d in range(ND)]
            wv_sb = [wp.tile([128, C], BF16, name=f"wv{d}") for d in range(ND)]
            wo_sb = [wp.tile([128, D], BF16, name=f"wo{t}") for t in range(NCC)]
            xt = [[xp.tile([128, 512], BF16, name=f"xt{lc}_{d}")
                   for d in range(ND)] for lc in range(NJ)]
            qT = [qkv.tile([128, L], BF16, name=f"qT{t}") for t in range(NCC)]
            kT = [qkv.tile([128, L], BF16, name=f"kT{t}") for t in range(NCC)]
            vp = [qkv.tile([128, HG, KEY_DIM + 1], BF16, name=f"vp{i}")
                  for i in range(16)]
            OF = [ofp.tile([128, L], BF16, name=f"of{t}") for t in range(NCC)]

            # input DMA, ordered so the first projection chain (wq + x lc=0)
            # can start as early as possible
            ls0 = slice(0, 512)
            for d in range(ND):
                nc.sync.dma_start(wq_sb[d][:], wq[128 * d:128 * (d + 1), :])
                nc.sync.dma_start(xt[0][d][:], xT[128 * d:128 * (d + 1), ls0])
            for d in range(ND):
                nc.sync.dma_start(wk_sb[d][:], wk[128 * d:128 * (d + 1), :])
            for d in range(ND):
                nc.sync.dma_start(wv_sb[d][:], wv[128 * d:128 * (d + 1), :])
            for lc in range(1, NJ):
                ls = slice(512 * lc, 512 * (lc + 1))
                for d in range(ND):
                    nc.sync.dma_start(xt[lc][d][:], xT[128 * d:128 * (d + 1), ls])
            for t in range(NCC):
                nc.sync.dma_start(wo_sb[t][:], wo[128 * t:128 * (t + 1), :])

            # ones column for the softmax-denominator trick (copies below
            # overwrite cols 0..63 of each head; col 64 stays 1.0)
            for i in range(16):
                nc.vector.memset(vp[i][:], 1.0)

            # ---------- filler units (issued between attention matmuls) ----
            def proj_chain(lc, c):
                """Chain c of 12 for l-chunk lc: c in 0..7 -> q/k interleaved
                (wq cc, wk cc, ...), c in 8..11 -> v l-subchunk."""
                ls = slice(512 * lc, 512 * (lc + 1))
                ps = psX.tile([128, 512], F32, name=f"pj{lc}_{c}", tag="psX")
                if c < 8:
                    w_sb, dst = ((wq_sb, qT) if c % 2 == 0 else (wk_sb, kT))
                    cc = c // 2
                    for d in range(ND):
                        nc.tensor.matmul(
                            ps[:], w_sb[d][:, 128 * cc:128 * (cc + 1)],
                            xt[lc][d][:], start=(d == 0), stop=(d == ND - 1))
                    cp = nc.scalar.copy if lc == 0 else nc.vector.tensor_copy
                    cp(dst[cc][:, ls], ps[:])
                else:
                    lcc = c - 8
                    i = 4 * lc + lcc
                    for d in range(ND):
                        nc.tensor.matmul(
                            ps[:], xt[lc][d][:, 128 * lcc:128 * (lcc + 1)],
                            wv_sb[d][:], start=(d == 0), stop=(d == ND - 1))
                    cp = nc.scalar.copy if lc == 0 else nc.vector.tensor_copy
                    cp(vp[i][:, :, 0:KEY_DIM],
                       ps[:].rearrange("p (h c) -> p h c", h=HG))

            def ph4_unit(qc, n):
                """Output projection for q-chunk qc (128 rows), d-half n."""
                qs = slice(128 * qc, 128 * (qc + 1))
                ns = slice(512 * n, 512 * (n + 1))
                f_ps = psX.tile([128, 512], F32, name=f"f{qc}_{n}", tag="psX")
                for t in range(NCC):
                    nc.tensor.matmul(f_ps[:], OF[t][:, qs], wo_sb[t][:, ns],
                                     start=(t == 0), stop=(t == NCC - 1))
                o_sb = osb.tile([128, 512], F32, name=f"ob{qc}_{n}", tag="osb")
                nc.vector.tensor_copy(o_sb[:], f_ps[:])
                nc.sync.dma_start(out[qs, ns], o_sb[:])

            # ---------- phase 0: projections for l-chunk 0 ----------------
            # wq chains first: they only need the wq + xt0 DMAs (issued
            # first); wk/wv DMAs stream in underneath them
            for c in (0, 2, 4, 6, 1, 3, 5, 7, 8, 9, 10, 11):
                proj_chain(0, c)

            # ---------- fused attention loop ------------------------------
            for j in range(NJ):
                js = slice(512 * j, 512 * (j + 1))
                n_i = 4 * j + 4
                fifo = []
                if j + 1 < NJ:
                    fifo += [("proj", j + 1, c) for c in range(12)]
                if j > 0:
                    fifo += [("ph4", qc, n)
                             for qc in range(4 * (j - 1), 4 * j)
                             for n in range(2)]

                def pop_filler(idx):
                    while fifo:
                        kind = fifo[0][0]
                        if kind == "ph4" and idx is not None and idx < 7:
                            return  # OF of prev block may not be ready yet
                        u = fifo.pop(0)
                        if u[0] == "proj":
                            proj_chain(u[1], u[2])
                        else:
                            ph4_unit(u[1], u[2])
                        return

                for hp in range(NCC):
                    p_tiles = []
                    o_ps = [psO.tile([65, 512], F32, name=f"o{j}{hp}{z}",
                                     tag="psO") for z in range(2)]

                    def pv(i):
                        # PV: accumulate [V|1]^T P^T for key chunk i
                        p_sb, off = p_tiles[i]
                        p3 = p_sb[:].rearrange("p (z w) -> p z w", z=2)
                        for z in range(2):
                            nc.tensor.matmul(
                                o_ps[z][:, off:512], vp[i][:, 2 * hp + z, :],
                                p3[:, z, off:512],
                                start=(i == 0), stop=(i == n_i - 1))

                    # S + exp (+ causal select), with the PV matmuls software-
                    # pipelined two chunks behind so the PE stream stays dense
                    # while exp (the ACT-side pacer) catches up
                    for i in range(n_i):
                        r = i - 4 * j  # >=0 on the diagonal blocks
                        off = 128 * r if r > 0 else 0
                        s_ps = psS.tile([128, 1024], F32,
                                        name=f"s{j}{hp}{i}", tag="psS")
                        for z in range(2):
                            rows = slice(64 * z, 64 * z + 64)
                            nc.tensor.matmul(
                                s_ps[:, 512 * z + off:512 * (z + 1)],
                                kT[hp][rows, 128 * i:128 * (i + 1)],
                                qT[hp][rows, 512 * j + off:512 * (j + 1)],
                                start=True, stop=True)
                        p_sb = pp.tile([128, 1024], BF16,
                                       name=f"p{j}{hp}{i}", tag="pp")
                        if off:
                            s3 = s_ps[:].rearrange("p (z w) -> p z w", z=2)
                            p3 = p_sb[:].rearrange("p (z w) -> p z w", z=2)
                            nc.scalar.activation(p3[:, :, off:512],
                                                 s3[:, :, off:512],
                                                 AF.Exp, scale=SCALE)
                        else:
                            nc.scalar.activation(p_sb[:], s_ps[:],
                                                 AF.Exp, scale=SCALE)
                        if r >= 0:
                            p3 = p_sb[:].rearrange("p (z w) -> p z w", z=2)
                            for z in range(2):
                                nc.gpsimd.affine_select(
                                    out=p3[:, z, off:off + 128],
                                    in_=p3[:, z, off:off + 128],
                                    compare_op=ALU.is_gt, fill=0.0,
                                    base=0, channel_multiplier=-1,
                                    pattern=[[1, 128]])
                        p_tiles.append((p_sb, off))
                        if i >= 2:
                            pv(i - 2)
                        if i % 2 == 1:
                            pop_filler(i)
                    for i in range(max(0, n_i - 2), n_i):
                        pv(i)
                    pop_filler(None)
                    # evacuate O', then normalize rows 0..63 by the sums row
                    # (row 64): ACT reciprocal -> Pool broadcast -> DVE scale
                    for z in range(2):
                        ov = ovp.tile([65, 512], F32, name=f"ov{j}{hp}{z}",
                                      tag="ovp")
                        nc.vector.tensor_copy(ov[:], o_ps[z][:])
                        if j == 0:
                            # q=0 is fully masked (sum 0) and host-patched;
                            # clamp into the ACT reciprocal's legal domain
                            nc.vector.tensor_scalar_max(
                                ov[64:65, :], ov[64:65, :], 2.3e-13)
                        lnr = bcp.tile([1, 512], F32, name=f"ln{j}{hp}{z}",
                                       tag="lnr")
                        r1 = bcp.tile([1, 512], F32, name=f"r1{j}{hp}{z}",
                                      tag="r1")
                        act_reciprocal(nc, lnr[:], r1[:], ov[64:65, :])
                        bc = bcp.tile([64, 512], F32, name=f"bc{j}{hp}{z}",
                                      tag="bcp")
                        nc.gpsimd.partition_broadcast(bc[:], r1[:])
                        nc.vector.tensor_tensor(
                            OF[hp][64 * z:64 * z + 64, js], ov[0:64, :],
                            bc[:], op=ALU.mult)
                while fifo:
                    pop_filler(None)

            # output projection for the last q block
            for qc in range(12, 16):
                for n in range(2):
                    ph4_unit(qc, n)

    nc.finalize()
    return nc


def _get_nc():
    if "nc" not in _CACHED:
        _CACHED["nc"] = build_nc()
    return _CACHED["nc"]


def kernel(x, W_q, W_k, W_v, W_out, trace=False, trace_kwargs=None):
    x = np.asarray(x, dtype=np.float32)
    W_q = np.asarray(W_q, dtype=np.float32)
    W_k = np.asarray(W_k, dtype=np.float32)
    W_v = np.asarray(W_v, dtype=np.float32)
    W_out = np.asarray(W_out, dtype=np.float32)
    bf = ml_dtypes.bfloat16

    nc = _get_nc()
    in_maps = []
    for core in range(8):
        b, g = core // 2, core % 2
        cs = slice(C * g, C * (g + 1))
        in_maps.append({
            "xT": np.ascontiguousarray(x[b].T.astype(bf)),
            "wq": np.ascontiguousarray(W_q[:, cs].astype(bf)),
            "wk": np.ascontiguousarray(W_k[:, cs].astype(bf)),
            "wv": np.ascontiguousarray(W_v[:, cs].astype(bf)),
            "wo": np.ascontiguousarray(W_out[cs, :].astype(bf)),
        })
    res = run_bass_kernel_spmd(nc, in_maps, core_ids=list(range(8)),
                               trace=trace, **(trace_kwargs or {}))
    out = np.empty((B, L, D), dtype=np.float32)
    for b in range(B):
        out[b] = res.results[2 * b]["out"] + res.results[2 * b + 1]["out"]
        # q=0 is fully masked -> reference softmax gives uniform attention over
        # all of V; the device leaves garbage in that row, patch it here.
        out[b, 0, :] = (x[b].mean(axis=0) @ W_v) @ W_out
    if trace:
        return out, res
    return out


# revision 26
# speedup vs baseline: 1.0116x; 1.0116x over previous
"""Multi-head causal attention (b=4, l=2048, d=1024, 16 heads x 64) on 8 trn2 cores.

Sharding: core c handles batch (c // 2) and head-group (c % 2) of 8 heads.
Each core computes a partial output x[b] @ W (its 8 heads' contribution);
the host sums the two partials per batch.

v2 design (vs fp32r baseline):
  - full bf16 datapath (PSUM stays fp32): enables PE fast-weight-load and
    halves DMA/SBUF traffic; accuracy is well within the 2e-2 gate.
  - single fused instruction stream: projections for l-chunk lc=j+1 and the
    output projection for q-block j-1 are interleaved (via a filler FIFO)
    into attention block j's S/PV matmuls, keeping the PE dense so it holds
    the fast HAM p-state and hides the exp (ACT) latency.
  - S tiles for the two heads of a pair land in one 2-bank PSUM tile and are
    exp'd with ONE activation instruction (halves ACT instruction overhead).
  - causal masking via gpsimd.affine_select on the diagonal 128x128 block of
    P (post-exp, fill=0); off-diagonal-block columns are skipped entirely
    (S / exp / PV all shortened to the unmasked column span).
  - softmax denominators via the [V|1] ones-column trick; normalization uses
    reciprocal_approx_fast (the exact DVE reciprocal is ~3.3us per call).

Device layouts (per core):
  xT      [1024, 2048]  x[b]^T (d on partitions), 32 sbuf tiles [128,512]
  wq/wk/wv [1024, 512]  head-group column slices; wo [512, 1024] row slice
  qT/kT   4 x [128, 2048]  c on partitions (head-pair per tile)
  vp      16 x [128, 8, 65]  v natural (l on partitions), per head 64 + ones
  OF      4 x [128, 2048]  normalized attention output (pre-out-proj)
  S^T     [128 m, 2x512 q] psum pairs -> exp -> P^T bf16; PV: [V|1]^T P^T
  q=0 is fully masked; its softmax sum is 0 and the garbage column is fixed
  up on the host (row 0 of the output only).
"""

import sys

sys.path.insert(0, "/opt/trn_rl_repo")

import ml_dtypes
import numpy as np

import concourse.bacc as bacc
import concourse.mybir as mybir
import concourse.tile as tile
from concourse.bass_utils import run_bass_kernel_spmd

F32 = mybir.dt.float32
BF16 = mybir.dt.bfloat16
AF = mybir.ActivationFunctionType
ALU = mybir.AluOpType

B, L, D = 4, 2048, 1024
N_HEAD, KEY_DIM = 16, 64
HG = 8               # heads per core (head-group)
C = HG * KEY_DIM     # 512 per-core qkv width
SCALE = 1.0 / 8.0    # 1/sqrt(KEY_DIM)
ND = 8               # d chunks of 128
NJ = 4               # q blocks of 512
NCC = 4              # c chunks of 128 (= head pairs)

_CACHED = {}


def act_reciprocal(nc, scratch, out, in_):
    """Reciprocal on the ACT engine as exp(-ln(x)).

    Ln and Exp share one activation-table set (natural_log_exp_and_others),
    so this never triggers a 1.3us ACT_TABLE_LOAD — unlike func=Reciprocal,
    whose table set excludes Exp and thrashes the table on every call. The
    DVE alternatives are unusable here: exact `reciprocal` costs ~3.3us per
    call (free-size-bound iterative), and `reciprocal_approx_fast`
    (custom-DVE uop table) returns garbage on HW via this compile path.
    """
    nc.scalar.activation(scratch, in_, AF.Ln)
    nc.scalar.activation(out, scratch, AF.Exp, scale=-1.0)


def build_nc():
    # NOTE: forcing all activations onto the shared
    # 'natural_log_exp_and_others' table set (one ACT_TABLE_LOAD instead of
    # ~35) measured ~339us but produced NaNs on HW — the table-load delays
    # evidently mask a timing-sensitive hand-off in the reciprocal path.
    # Keeping the stock (thrashing but correct) table schedule.
    nc = bacc.Bacc("TRN2", target_bir_lowering=False, debug=False)

    xT = nc.dram_tensor("xT", [D, L], BF16, kind="ExternalInput")
    wq = nc.dram_tensor("wq", [D, C], BF16, kind="ExternalInput")
    wk = nc.dram_tensor("wk", [D, C], BF16, kind="ExternalInput")
    wv = nc.dram_tensor("wv", [D, C], BF16, kind="ExternalInput")
    wo = nc.dram_tensor("wo", [C, D], BF16, kind="ExternalInput")
    out = nc.dram_tensor("out", [L, D], F32, kind="ExternalOutput")

    with tile.TileContext(nc) as tc:
        with tc.tile_pool(name="wp", bufs=1) as wp, \
             tc.tile_pool(name="xp", bufs=1) as xp, \
             tc.tile_pool(name="qkv", bufs=1) as qkv, \
             tc.tile_pool(name="ofp", bufs=1) as ofp, \
             tc.tile_pool(name="pp", bufs=6) as pp, \
             tc.tile_pool(name="ovp", bufs=4) as ovp, \
             tc.tile_pool(name="bcp", bufs=2) as bcp, \
             tc.tile_pool(name="osb", bufs=2) as osb, \
             tc.tile_pool(name="psS", bufs=2, space="PSUM") as psS, \
             tc.tile_pool(name="psX", bufs=1, space="PSUM") as psX, \
             tc.tile_pool(name="psO", bufs=3, space="PSUM") as psO:

            wq_sb = [wp.tile([128, C], BF16, name=f"wq{d}") for d in range(ND)]
            wk_sb = [wp.tile([128, C], BF16, name=f"wk{d}") for d in range(ND)]
            wv_sb = [wp.tile([128, C], BF16, name=f"wv{d}") for d in range(ND)]
            wo_sb = [wp.tile([128, D], BF16, name=f"wo{t}") for t in range(NCC)]
            xt = [[xp.tile([128, 512], BF16, name=f"xt{lc}_{d}")
                   for d in range(ND)] for lc in range(NJ)]
            qT = [qkv.tile([128, L], BF16, name=f"qT{t}") for t in range(NCC)]
            kT = [qkv.tile([128, L], BF16, name=f"kT{t}") for t in range(NCC)]
            vp = [qkv.tile([128, HG, KEY_DIM + 1], BF16, name=f"vp{i}")
                  for i in range(16)]
            OF = [ofp.tile([128, L], BF16, name=f"of{t}") for t in range(NCC)]

            # input DMA, ordered so the first projection chain (wq + x lc=0)
            # can start as early as possible
            ls0 = slice(0, 512)
            for d in range(ND):
                nc.sync.dma_start(wq_sb[d][:], wq[128 * d:128 * (d + 1), :])
                nc.sync.dma_start(xt[0][d][:], xT[128 * d:128 * (d + 1), ls0])
            for d in range(ND):
                nc.sync.dma_start(wk_sb[d][:], wk[128 * d:128 * (d + 1), :])
            for d in range(ND):
                nc.sync.dma_start(wv_sb[d][:], wv[128 * d:128 * (d + 1), :])
            for lc in range(1, NJ):
                ls = slice(512 * lc, 512 * (lc + 1))
                for d in range(ND):
                    nc.sync.dma_start(xt[lc][d][:], xT[128 * d:128 * (d + 1), ls])
            for t in range(NCC):
                nc.sync.dma_start(wo_sb[t][:], wo[128 * t:128 * (t + 1), :])

            # ones column for the softmax-denominator trick (copies below
            # overwrite cols 0..63 of each head; col 64 stays 1.0)
            for i in range(16):
                nc.vector.memset(vp[i][:], 1.0)

            # ---------- filler units (issued between attention matmuls) ----
            def proj_chain(lc, c):
                """Chain c of 12 for l-chunk lc: c in 0..7 -> q/k interleaved
                (wq cc, wk cc, ...), c in 8..11 -> v l-subchunk."""
                ls = slice(512 * lc, 512 * (lc + 1))
                ps = psX.tile([128, 512], F32, name=f"pj{lc}_{c}", tag="psX")
                if c < 8:
                    w_sb, dst = ((wq_sb, qT) if c % 2 == 0 else (wk_sb, kT))
                    cc = c // 2
                    for d in range(ND):
                        nc.tensor.matmul(
                            ps[:], w_sb[d][:, 128 * cc:128 * (cc + 1)],
                            xt[lc][d][:], start=(d == 0), stop=(d == ND - 1))
                    cp = nc.scalar.copy if lc == 0 else nc.vector.tensor_copy
                    cp(dst[cc][:, ls], ps[:])
                else:
                    lcc = c - 8
                    i = 4 * lc + lcc
                    for d in range(ND):
                        nc.tensor.matmul(
                            ps[:], xt[lc][d][:, 128 * lcc:128 * (lcc + 1)],
                            wv_sb[d][:], start=(d == 0), stop=(d == ND - 1))
                    cp = nc.scalar.copy if lc == 0 else nc.vector.tensor_copy
                    cp(vp[i][:, :, 0:KEY_DIM],
                       ps[:].rearrange("p (h c) -> p h c", h=HG))

            def ph4_unit(qc, n):
                """Output projection for q-chunk qc (128 rows), d-half n."""
                qs = slice(128 * qc, 128 * (qc + 1))
                ns = slice(512 * n, 512 * (n + 1))
                f_ps = psX.tile([128, 512], F32, name=f"f{qc}_{n}", tag="psX")
                for t in range(NCC):
                    nc.tensor.matmul(f_ps[:], OF[t][:, qs], wo_sb[t][:, ns],
                                     start=(t == 0), stop=(t == NCC - 1))
                o_sb = osb.tile([128, 512], F32, name=f"ob{qc}_{n}", tag="osb")
                nc.vector.tensor_copy(o_sb[:], f_ps[:])
                nc.sync.dma_start(out[qs, ns], o_sb[:])

            # ---------- phase 0: projections for l-chunk 0 ----------------
            # wq chains first: they only need the wq + xt0 DMAs (issued
            # first); wk/wv DMAs stream in underneath them
            for c in (0, 2, 4, 6, 1, 3, 5, 7, 8, 9, 10, 11):
                proj_chain(0, c)

            # ---------- fused attention loop ------------------------------
            for j in range(NJ):
                js = slice(512 * j, 512 * (j + 1))
                n_i = 4 * j + 4
                fifo = []
                if j + 1 < NJ:
                    fifo += [("proj", j + 1, c) for c in range(12)]
                if j > 0:
                    fifo += [("ph4", qc, n)
                             for qc in range(4 * (j - 1), 4 * j)
                             for n in range(2)]

                def pop_filler(idx):
                    while fifo:
                        kind = fifo[0][0]
                        if kind == "ph4" and idx is not None and idx < 7:
                            return  # OF of prev block may not be ready yet
                        u = fifo.pop(0)
                        if u[0] == "proj":
                            proj_chain(u[1], u[2])
                        else:
                            ph4_unit(u[1], u[2])
                        return

                for hp in range(NCC):
                    p_tiles = []
                    o_ps = [psO.tile([65, 512], F32, name=f"o{j}{hp}{z}",
                                     tag="psO") for z in range(2)]

                    def pv(i):
                        # PV: accumulate [V|1]^T P^T for key chunk i
                        p_sb, off = p_tiles[i]
                        p3 = p_sb[:].rearrange("p (z w) -> p z w", z=2)
                        for z in range(2):
                            nc.tensor.matmul(
                                o_ps[z][:, off:512], vp[i][:, 2 * hp + z, :],
                                p3[:, z, off:512],
                                start=(i == 0), stop=(i == n_i - 1))

                    # S + exp (+ causal select), with the PV matmuls software-
                    # pipelined two chunks behind so the PE stream stays dense
                    # while exp (the ACT-side pacer) catches up
                    for i in range(n_i):
                        r = i - 4 * j  # >=0 on the diagonal blocks
                        off = 128 * r if r > 0 else 0
                        s_ps = psS.tile([128, 1024], F32,
                                        name=f"s{j}{hp}{i}", tag="psS")
                        for z in range(2):
                            rows = slice(64 * z, 64 * z + 64)
                            nc.tensor.matmul(
                                s_ps[:, 512 * z + off:512 * (z + 1)],
                                kT[hp][rows, 128 * i:128 * (i + 1)],
                                qT[hp][rows, 512 * j + off:512 * (j + 1)],
                                start=True, stop=True)
                        p_sb = pp.tile([128, 1024], BF16,
                                       name=f"p{j}{hp}{i}", tag="pp")
                        if off:
                            s3 = s_ps[:].rearrange("p (z w) -> p z w", z=2)
                            p3 = p_sb[:].rearrange("p (z w) -> p z w", z=2)
                            nc.scalar.activation(p3[:, :, off:512],
                                                 s3[:, :, off:512],
                                                 AF.Exp, scale=SCALE)
                        else:
                            nc.scalar.activation(p_sb[:], s_ps[:],
                                                 AF.Exp, scale=SCALE)
                        if r >= 0:
                            p3 = p_sb[:].rearrange("p (z w) -> p z w", z=2)
                            for z in range(2):
                                nc.gpsimd.affine_select(
                                    out=p3[:, z, off:off + 128],
                                    in_=p3[:, z, off:off + 128],
                                    compare_op=ALU.is_gt, fill=0.0,
                                    base=0, channel_multiplier=-1,
                                    pattern=[[1, 128]])
                        p_tiles.append((p_sb, off))
                        if i >= 2:
                            pv(i - 2)
                        if i % 2 == 1:
                            pop_filler(i)
                    for i in range(max(0, n_i - 2), n_i):
                        pv(i)
                    pop_filler(None)
                    # evacuate O', then normalize rows 0..63 by the sums row
                    # (row 64): ACT reciprocal -> Pool broadcast -> DVE scale
                    for z in range(2):
                        ov = ovp.tile([65, 512], F32, name=f"ov{j}{hp}{z}",
                                      tag="ovp")
                        nc.vector.tensor_copy(ov[:], o_ps[z][:])
                        if j == 0:
                            # q=0 is fully masked (sum 0) and host-patched;
                            # clamp into the ACT reciprocal's legal domain
                            nc.vector.tensor_scalar_max(
                                ov[64:65, :], ov[64:65, :], 2.3e-13)
                        lnr = bcp.tile([1, 512], F32, name=f"ln{j}{hp}{z}",
                                       tag="lnr")
                        r1 = bcp.tile([1, 512], F32, name=f"r1{j}{hp}{z}",
                                      tag="r1")
                        act_reciprocal(nc, lnr[:], r1[:], ov[64:65, :])
                        bc = bcp.tile([64, 512], F32, name=f"bc{j}{hp}{z}",
                                      tag="bcp")
                        nc.gpsimd.partition_broadcast(bc[:], r1[:])
                        nc.vector.tensor_tensor(
                            OF[hp][64 * z:64 * z + 64, js], ov[0:64, :],
                            bc[:], op=ALU.mult)
                while fifo:
                    pop_filler(None)

            # output projection for the last q block
            for qc in range(12, 16):
                for n in range(2):
                    ph4_unit(qc, n)

    nc.finalize()
    return nc


def _get_nc():
    if "nc" not in _CACHED:
        _CACHED["nc"] = build_nc()
    return _CACHED["nc"]


def kernel(x, W_q, W_k, W_v, W_out, trace=False, trace_kwargs=None):
    x = np.asarray(x, dtype=np.float32)
    W_q = np.asarray(W_q, dtype=np.float32)
    W_k = np.asarray(W_k, dtype=np.float32)
    W_v = np.asarray(W_v, dtype=np.float32)
    W_out = np.asarray(W_out, dtype=np.float32)
    bf = ml_dtypes.bfloat16

    nc = _get_nc()
    in_maps = []
    for core in range(8):
        b, g = core // 2, core % 2
        cs = slice(C * g, C * (g + 1))
        in_maps.append({
            "xT": np.ascontiguousarray(x[b].T.astype(bf)),
            "wq": np.ascontiguousarray(W_q[:, cs].astype(bf)),
            "wk": np.ascontiguousarray(W_k[:, cs].astype(bf)),
            "wv": np.ascontiguousarray(W_v[:, cs].astype(bf)),
            "wo": np.ascontiguousarray(W_out[cs, :].astype(bf)),
        })
    res = run_bass_kernel_spmd(nc, in_maps, core_ids=list(range(8)),
                               trace=trace, **(trace_kwargs or {}))
    out = np.empty((B, L, D), dtype=np.float32)
    for b in range(B):
        out[b] = res.results[2 * b]["out"] + res.results[2 * b + 1]["out"]
        # q=0 is fully masked -> reference softmax gives uniform attention over
        # all of V; the device leaves garbage in that row, patch it here.
        out[b, 0, :] = (x[b].mean(axis=0) @ W_v) @ W_out
    if trace:
        return out, res
    return out


# revision 30
# speedup vs baseline: 1.1281x; 1.1152x over previous
"""Multi-head causal attention (b=4, l=2048, d=1024, 16 heads x 64) on 8 trn2 cores.

Sharding: core c handles batch (c // 2) and head-group (c % 2) of 8 heads.
Each core computes a partial output x[b] @ W (its 8 heads' contribution);
the host sums the two partials per batch.

v2 design (vs fp32r baseline):
  - full bf16 datapath (PSUM stays fp32): enables PE fast-weight-load and
    halves DMA/SBUF traffic; accuracy is well within the 2e-2 gate.
  - single fused instruction stream: projections for l-chunk lc=j+1 and the
    output projection for q-block j-1 are interleaved (via a filler FIFO)
    into attention block j's S/PV matmuls, keeping the PE dense so it holds
    the fast HAM p-state and hides the exp (ACT) latency.
  - S tiles for the two heads of a pair land in one 2-bank PSUM tile and are
    exp'd with ONE activation instruction (halves ACT instruction overhead).
  - causal masking via gpsimd.affine_select on the diagonal 128x128 block of
    P (post-exp, fill=0); off-diagonal-block columns are skipped entirely
    (S / exp / PV all shortened to the unmasked column span).
  - softmax denominators via the [V|1] ones-column trick; normalization uses
    reciprocal_approx_fast (the exact DVE reciprocal is ~3.3us per call).

Device layouts (per core):
  xT      [1024, 2048]  x[b]^T (d on partitions), 32 sbuf tiles [128,512]
  wq/wk/wv [1024, 512]  head-group column slices; wo [512, 1024] row slice
  qT/kT   4 x [128, 2048]  c on partitions (head-pair per tile)
  vp      16 x [128, 8, 65]  v natural (l on partitions), per head 64 + ones
  OF      4 x [128, 2048]  normalized attention output (pre-out-proj)
  S^T     [128 m, 2x512 q] psum pairs -> exp -> P^T bf16; PV: [V|1]^T P^T
  q=0 is fully masked; its softmax sum is 0 and the garbage column is fixed
  up on the host (row 0 of the output only).
"""

import sys

sys.path.insert(0, "/opt/trn_rl_repo")

import ml_dtypes
import numpy as np

import concourse.bacc as bacc
import concourse.mybir as mybir
import concourse.tile as tile
from concourse.bass_utils import run_bass_kernel_spmd

F32 = mybir.dt.float32
BF16 = mybir.dt.bfloat16
AF = mybir.ActivationFunctionType
ALU = mybir.AluOpType

B, L, D = 4, 2048, 1024
N_HEAD, KEY_DIM = 16, 64
HG = 8               # heads per core (head-group)
C = HG * KEY_DIM     # 512 per-core qkv width
SCALE = 1.0 / 8.0    # 1/sqrt(KEY_DIM)
ND = 8               # d chunks of 128
NJ = 4               # q blocks of 512
NCC = 4              # c chunks of 128 (= head pairs)

_CACHED = {}


def build_nc():
    # NOTE: forcing all activations onto the shared
    # 'natural_log_exp_and_others' table set (one ACT_TABLE_LOAD instead of
    # ~35) measured ~339us but produced NaNs on HW — the table-load delays
    # evidently mask a timing-sensitive hand-off in the reciprocal path.
    # Keeping the stock (thrashing but correct) table schedule.
    nc = bacc.Bacc("TRN2", target_bir_lowering=False, debug=False)

    xT = nc.dram_tensor("xT", [D, L], BF16, kind="ExternalInput")
    wq = nc.dram_tensor("wq", [D, C], BF16, kind="ExternalInput")
    wk = nc.dram_tensor("wk", [D, C], BF16, kind="ExternalInput")
    wv = nc.dram_tensor("wv", [D, C], BF16, kind="ExternalInput")
    wo = nc.dram_tensor("wo", [C, D], BF16, kind="ExternalInput")
    out = nc.dram_tensor("out", [L, D], F32, kind="ExternalOutput")

    with tile.TileContext(nc) as tc:
        with tc.tile_pool(name="wp", bufs=1) as wp, \
             tc.tile_pool(name="xp", bufs=1) as xp, \
             tc.tile_pool(name="qkv", bufs=1) as qkv, \
             tc.tile_pool(name="ofp", bufs=1) as ofp, \
             tc.tile_pool(name="pp", bufs=6) as pp, \
             tc.tile_pool(name="ovp", bufs=9) as ovp, \
             tc.tile_pool(name="smp", bufs=1) as smp, \
             tc.tile_pool(name="bcp", bufs=2) as bcp, \
             tc.tile_pool(name="osb", bufs=2) as osb, \
             tc.tile_pool(name="psS", bufs=2, space="PSUM") as psS, \
             tc.tile_pool(name="psX", bufs=1, space="PSUM") as psX, \
             tc.tile_pool(name="psO", bufs=3, space="PSUM") as psO:

            wq_sb = [wp.tile([128, C], BF16, name=f"wq{d}") for d in range(ND)]
            wk_sb = [wp.tile([128, C], BF16, name=f"wk{d}") for d in range(ND)]
            wv_sb = [wp.tile([128, C], BF16, name=f"wv{d}") for d in range(ND)]
            wo_sb = [wp.tile([128, D], BF16, name=f"wo{t}") for t in range(NCC)]
            xt = [[xp.tile([128, 512], BF16, name=f"xt{lc}_{d}")
                   for d in range(ND)] for lc in range(NJ)]
            qT = [qkv.tile([128, L], BF16, name=f"qT{t}") for t in range(NCC)]
            kT = [qkv.tile([128, L], BF16, name=f"kT{t}") for t in range(NCC)]
            vp = [qkv.tile([128, HG, KEY_DIM + 1], BF16, name=f"vp{i}")
                  for i in range(16)]
            OF = [ofp.tile([128, L], BF16, name=f"of{t}") for t in range(NCC)]

            # input DMA, ordered so the first projection chain (wq + x lc=0)
            # can start as early as possible
            ls0 = slice(0, 512)
            for d in range(ND):
                nc.sync.dma_start(wq_sb[d][:], wq[128 * d:128 * (d + 1), :])
                nc.sync.dma_start(xt[0][d][:], xT[128 * d:128 * (d + 1), ls0])
            for d in range(ND):
                nc.sync.dma_start(wk_sb[d][:], wk[128 * d:128 * (d + 1), :])
            for d in range(ND):
                nc.sync.dma_start(wv_sb[d][:], wv[128 * d:128 * (d + 1), :])
            for lc in range(1, NJ):
                ls = slice(512 * lc, 512 * (lc + 1))
                for d in range(ND):
                    nc.sync.dma_start(xt[lc][d][:], xT[128 * d:128 * (d + 1), ls])
            for t in range(NCC):
                nc.sync.dma_start(wo_sb[t][:], wo[128 * t:128 * (t + 1), :])

            # ones column for the softmax-denominator trick (copies below
            # overwrite cols 0..63 of each head; col 64 stays 1.0)
            for i in range(16):
                nc.vector.memset(vp[i][:], 1.0)

            # ---------- filler units (issued between attention matmuls) ----
            def proj_chain(lc, c):
                """Chain c of 12 for l-chunk lc: c in 0..7 -> q/k interleaved
                (wq cc, wk cc, ...), c in 8..11 -> v l-subchunk."""
                ls = slice(512 * lc, 512 * (lc + 1))
                ps = psX.tile([128, 512], F32, name=f"pj{lc}_{c}", tag="psX")
                if c < 8:
                    w_sb, dst = ((wq_sb, qT) if c % 2 == 0 else (wk_sb, kT))
                    cc = c // 2
                    for d in range(ND):
                        nc.tensor.matmul(
                            ps[:], w_sb[d][:, 128 * cc:128 * (cc + 1)],
                            xt[lc][d][:], start=(d == 0), stop=(d == ND - 1))
                    cp = nc.scalar.copy if lc == 0 else nc.vector.tensor_copy
                    cp(dst[cc][:, ls], ps[:])
                else:
                    lcc = c - 8
                    i = 4 * lc + lcc
                    for d in range(ND):
                        nc.tensor.matmul(
                            ps[:], xt[lc][d][:, 128 * lcc:128 * (lcc + 1)],
                            wv_sb[d][:], start=(d == 0), stop=(d == ND - 1))
                    cp = nc.scalar.copy if lc == 0 else nc.vector.tensor_copy
                    cp(vp[i][:, :, 0:KEY_DIM],
                       ps[:].rearrange("p (h c) -> p h c", h=HG))

            def ph4_unit(qc, n):
                """Output projection for q-chunk qc (128 rows), d-half n."""
                qs = slice(128 * qc, 128 * (qc + 1))
                ns = slice(512 * n, 512 * (n + 1))
                f_ps = psX.tile([128, 512], F32, name=f"f{qc}_{n}", tag="psX")
                for t in range(NCC):
                    nc.tensor.matmul(f_ps[:], OF[t][:, qs], wo_sb[t][:, ns],
                                     start=(t == 0), stop=(t == NCC - 1))
                o_sb = osb.tile([128, 512], F32, name=f"ob{qc}_{n}", tag="osb")
                nc.vector.tensor_copy(o_sb[:], f_ps[:])
                nc.sync.dma_start(out[qs, ns], o_sb[:])

            # ---------- phase 0: projections for l-chunk 0 ----------------
            # wq chains first: they only need the wq + xt0 DMAs (issued
            # first); wk/wv DMAs stream in underneath them
            for c in (0, 2, 4, 6, 1, 3, 5, 7, 8, 9, 10, 11):
                proj_chain(0, c)

            # ---------- fused attention loop ------------------------------
            for j in range(NJ):
                js = slice(512 * j, 512 * (j + 1))
                n_i = 4 * j + 4
                fifo = []
                if j + 1 < NJ:
                    fifo += [("proj", j + 1, c) for c in range(12)]
                if j > 0:
                    fifo += [("ph4", qc, n)
                             for qc in range(4 * (j - 1), 4 * j)
                             for n in range(2)]

                def pop_filler(idx):
                    while fifo:
                        kind = fifo[0][0]
                        if kind == "ph4" and idx is not None and idx < 7:
                            return  # OF of prev block may not be ready yet
                        u = fifo.pop(0)
                        if u[0] == "proj":
                            proj_chain(u[1], u[2])
                        else:
                            ph4_unit(u[1], u[2])
                        return

                sums = [smp.tile([128, 512], F32, name=f"sums{j}{t}",
                                 tag=f"sums{t}") for t in range(2)]
                for t in range(2):
                    nc.gpsimd.memset(sums[t][:], 1.0)
                ovs = {}

                for hp in range(NCC):
                    p_tiles = []
                    o_ps = [psO.tile([65, 512], F32, name=f"o{j}{hp}{z}",
                                     tag="psO") for z in range(2)]

                    def pv(i):
                        # PV: accumulate [V|1]^T P^T for key chunk i
                        p_sb, off = p_tiles[i]
                        p3 = p_sb[:].rearrange("p (z w) -> p z w", z=2)
                        for z in range(2):
                            nc.tensor.matmul(
                                o_ps[z][:, off:512], vp[i][:, 2 * hp + z, :],
                                p3[:, z, off:512],
                                start=(i == 0), stop=(i == n_i - 1))

                    # S + exp (+ causal select), with the PV matmuls software-
                    # pipelined two chunks behind so the PE stream stays dense
                    # while exp (the ACT-side pacer) catches up
                    for i in range(n_i):
                        r = i - 4 * j  # >=0 on the diagonal blocks
                        off = 128 * r if r > 0 else 0
                        s_ps = psS.tile([128, 1024], F32,
                                        name=f"s{j}{hp}{i}", tag="psS")
                        for z in range(2):
                            rows = slice(64 * z, 64 * z + 64)
                            nc.tensor.matmul(
                                s_ps[:, 512 * z + off:512 * (z + 1)],
                                kT[hp][rows, 128 * i:128 * (i + 1)],
                                qT[hp][rows, 512 * j + off:512 * (j + 1)],
                                start=True, stop=True)
                        p_sb = pp.tile([128, 1024], BF16,
                                       name=f"p{j}{hp}{i}", tag="pp")
                        if off:
                            s3 = s_ps[:].rearrange("p (z w) -> p z w", z=2)
                            p3 = p_sb[:].rearrange("p (z w) -> p z w", z=2)
                            nc.scalar.activation(p3[:, :, off:512],
                                                 s3[:, :, off:512],
                                                 AF.Exp, scale=SCALE)
                        else:
                            nc.scalar.activation(p_sb[:], s_ps[:],
                                                 AF.Exp, scale=SCALE)
                        if r >= 0:
                            p3 = p_sb[:].rearrange("p (z w) -> p z w", z=2)
                            for z in range(2):
                                nc.gpsimd.affine_select(
                                    out=p3[:, z, off:off + 128],
                                    in_=p3[:, z, off:off + 128],
                                    compare_op=ALU.is_gt, fill=0.0,
                                    base=0, channel_multiplier=-1,
                                    pattern=[[1, 128]])
                        p_tiles.append((p_sb, off))
                        if i >= 2:
                            pv(i - 2)
                        if i % 2 == 1:
                            pop_filler(i)
                    for i in range(max(0, n_i - 2), n_i):
                        pv(i)
                    pop_filler(None)
                    # evacuate O'; sums rows (row 64) collect into shared
                    # tiles at quadrant-aligned partitions (0/32/64/96) —
                    # the exact DVE reciprocal is free-size-bound (~3.3us per
                    # call regardless of partitions), so batching 4 rows per
                    # tile amortizes it to 2 calls per q-block
                    for z in range(2):
                        flat = 2 * hp + z
                        ov = ovp.tile([65, 512], F32, name=f"ov{j}{hp}{z}",
                                      tag="ovp")
                        nc.vector.tensor_copy(ov[:], o_ps[z][:])
                        row = 32 * (flat % 4)
                        nc.vector.tensor_copy(
                            sums[flat // 4][row:row + 1, :], ov[64:65, :])
                        ovs[(hp, z)] = ov
                rec = [smp.tile([128, 512], F32, name=f"rec{j}{t}",
                                tag=f"rec{t}") for t in range(2)]
                for t in range(2):
                    nc.vector.reciprocal(rec[t][:], sums[t][:])
                for hp in range(NCC):
                    for z in range(2):
                        flat = 2 * hp + z
                        row = 32 * (flat % 4)
                        # stage the quadrant row down to partition 0
                        # (partition_broadcast requires base-0 input)
                        r1 = bcp.tile([1, 512], F32, name=f"r1{j}{hp}{z}",
                                      tag="r1")
                        nc.vector.tensor_copy(
                            r1[:], rec[flat // 4][row:row + 1, :])
                        bc = bcp.tile([64, 512], F32, name=f"bc{j}{hp}{z}",
                                      tag="bcp")
                        nc.gpsimd.partition_broadcast(bc[:], r1[:])
                        nc.vector.tensor_tensor(
                            OF[hp][64 * z:64 * z + 64, js],
                            ovs[(hp, z)][0:64, :], bc[:], op=ALU.mult)
                while fifo:
                    pop_filler(None)

            # output projection for the last q block
            for qc in range(12, 16):
                for n in range(2):
                    ph4_unit(qc, n)

    nc.finalize()
    return nc


def _get_nc():
    if "nc" not in _CACHED:
        _CACHED["nc"] = build_nc()
    return _CACHED["nc"]


def kernel(x, W_q, W_k, W_v, W_out, trace=False, trace_kwargs=None):
    x = np.asarray(x, dtype=np.float32)
    W_q = np.asarray(W_q, dtype=np.float32)
    W_k = np.asarray(W_k, dtype=np.float32)
    W_v = np.asarray(W_v, dtype=np.float32)
    W_out = np.asarray(W_out, dtype=np.float32)
    bf = ml_dtypes.bfloat16

    nc = _get_nc()
    in_maps = []
    for core in range(8):
        b, g = core // 2, core % 2
        cs = slice(C * g, C * (g + 1))
        in_maps.append({
            "xT": np.ascontiguousarray(x[b].T.astype(bf)),
            "wq": np.ascontiguousarray(W_q[:, cs].astype(bf)),
            "wk": np.ascontiguousarray(W_k[:, cs].astype(bf)),
            "wv": np.ascontiguousarray(W_v[:, cs].astype(bf)),
            "wo": np.ascontiguousarray(W_out[cs, :].astype(bf)),
        })
    res = run_bass_kernel_spmd(nc, in_maps, core_ids=list(range(8)),
                               trace=trace, **(trace_kwargs or {}))
    out = np.empty((B, L, D), dtype=np.float32)
    for b in range(B):
        out[b] = res.results[2 * b]["out"] + res.results[2 * b + 1]["out"]
        # q=0 is fully masked -> reference softmax gives uniform attention over
        # all of V; the device leaves garbage in that row, patch it here.
        out[b, 0, :] = (x[b].mean(axis=0) @ W_v) @ W_out
    if trace:
        return out, res
    return out
